# revision 27
# baseline (speedup 1.0000x reference)
"""Trainium2 Bass kernel for a 3-layer LSTM (B=4096, T=1024, IN=2, H=16) + final FC.

Per core (batch-sharded 8 ways, B_local=512), wavefront over layers:
macro-step s computes L0@t=s, L1@t=s-1, L2@t=s-2. The batch is further split
into 2 staggered chunks of 256 so two independent recurrence chains can
interleave across engines (hides the per-step dependency-chain latency).

v2 changes vs baseline (4.29ms):
  - x is read DIRECTLY by the PE as a second accumulating K=2 matmul per
    PSUM bank (tile_position=(0,0)), killing the per-step SBUF->SBUF x-stage
    DMA that sat on the recurrence critical path (SP seq ~1.4us per step).
  - ONE merged sigmoid per chunk-step over both PSUM banks (FD=512) instead
    of two FD=256 ops (saves one ACT fixed cost per chunk-step).
  - Cell state is stored as c/2 ("C-half"): then c'/2 = HU + FW is a plain
    2x-mode tensor_tensor ADD (194ns) instead of a 1x STT (327ns), and
    tanh(c') = tanh(2 * C_half) uses ACT's free scale=2 affine.
  - FW = f*C_half runs on the (otherwise idle) Pool engine to keep DVE
    under the ACT roofline.

Stationary weights WT [51, 256] f16: rows 0:48 = h0,h1,h2 recurrent/inter-
layer weights, row 48 = bias, rows 49:51 = x weights (layer 0 only).
Moving window S [49, CB]: rows 0:48 h, row 48 = ones (bias).
Gate cols per 128-half: first group (i or 2g) at 16*l, second (f or o) at
64+16*l; g's weights+bias prescaled x2 so sigmoid(2g) = (tanh(g)+1)/2.

Per chunk-step:
  MM_Ah (K=49, start) + MM_Ax (K=2, stop)  -> PSUM bank A rows: i(0:48) f(64:112)
  MM_Bh + MM_Bx                            -> PSUM bank B rows: 2g, o
  SIG  G[128,2,CB] = sigmoid(P)            ACT, one op FD=2*CB
  FW   = G_f * C_half                      Pool tensor_mul
  HU   = (G_2g - 0.5) * G_i                DVE STT  [= i*tanh(g)/2]
  C_half' = HU + FW                        DVE tensor_add (2x mode)
  TC   = tanh(2 * C_half')                 ACT scale=2
  S[0:48] = G_o * TC  (h0,h1,h2 at once)   DVE tensor_mul
"""

import os
import sys

sys.path.insert(0, "/opt/trn_rl_repo")

import numpy as np

import concourse.bacc as bacc
import concourse.mybir as mybir
from concourse.tile import TileContext
from concourse import bass_utils

B, T, IN, H, L = 4096, 1024, 2, 16, 3
NCORES = 8
BL = B // NCORES          # 512
NCHUNK = int(os.environ.get("LSTM_NCHUNK", "3"))
if BL % NCHUNK == 0:
    CBS = [BL // NCHUNK] * NCHUNK
else:
    # uneven chunks (e.g. NCHUNK=3 -> 172,172,168); keep sizes even for
    # DVE 2x mode
    base = (BL // NCHUNK) // 2 * 2
    CBS = [base + 2] * ((BL - base * NCHUNK) // 2)
    CBS += [base] * (NCHUNK - len(CBS))
    assert sum(CBS) == BL and all(c % 2 == 0 for c in CBS), CBS
OFFS = [sum(CBS[:k]) for k in range(NCHUNK)]
F32 = mybir.dt.float32
F16 = mybir.dt.float16
NPF16 = np.float16

# PyTorch gate rows in W_ih*/W_hh*: i, f, g, o
PT_I, PT_F, PT_G, PT_O = slice(0, 16), slice(16, 32), slice(32, 48), slice(48, 64)

_STEPS_ENV = int(os.environ.get("LSTM_STEPS", "0"))
_FW_POOL = os.environ.get("LSTM_FW_POOL", "0") == "1"
_CDT_ENV = os.environ.get("LSTM_CDT", "f16")  # cell-state dtype: f32 | f16


def _t_run():
    return _STEPS_ENV if _STEPS_ENV > 0 else T


KH = 49  # h rows + bias row
X0 = 64  # x rows base: must be 32-aligned for engine (DVE) partition access
KW = 66  # total stationary/moving contraction rows (h + bias + pad + 2 x)


def build_weight_block(W_ih0, W_hh0, b0, W_ih1, W_hh1, b1, W_ih2, W_hh2, b2):
    """WT [51, 256] f32. Cols 0:128 = bank A (i,f), 128:256 = bank B (2g, o).

    K rows: 0:16 h0, 16:32 h1, 32:48 h2, 48 one(bias), 49:51 x.
    Col layout within each bank: layer-l gate block at 16*l : 16*l+16 for
    the first gate group (i or g), 64+16*l : 64+16*l+16 for the second (f or o).
    """
    WT = np.zeros((KW, 256), np.float32)
    layers = [
        # (in_rows, rec_rows, W_ih, W_hh, b)
        (slice(X0, X0 + 2), slice(0, 16), W_ih0, W_hh0, b0),
        (slice(0, 16), slice(16, 32), W_ih1, W_hh1, b1),
        (slice(16, 32), slice(32, 48), W_ih2, W_hh2, b2),
    ]
    for half, (pt0, pt1) in ((0, (PT_I, PT_F)), (128, (PT_G, PT_O))):
        for l, (ir, rr, Wih, Whh, b) in enumerate(layers):
            for pt, cbase, sc in ((pt0, half + 16 * l, 2.0 if half else 1.0),
                                  (pt1, half + 64 + 16 * l, 1.0)):
                WT[ir, cbase:cbase + 16] = sc * Wih[pt].T
                WT[rr, cbase:cbase + 16] = sc * Whh[pt].T
                WT[48, cbase:cbase + 16] = sc * b[pt]
    return WT


def build_bass():
    CDT = F32 if _CDT_ENV == "f32" else F16
    NPC = np.float32 if _CDT_ENV == "f32" else NPF16
    nc = bacc.Bacc("TRN2", target_bir_lowering=False, debug=False,
                   num_devices=NCORES)
    NT = _t_run() + 2
    nblk = (_t_run() + 63) // 64

    wt_d = nc.dram_tensor("wt", [KW, 256], F16, kind="ExternalInput")
    wfc_d = nc.dram_tensor("wfc", [17, 1], F16, kind="ExternalInput")
    xt_d, s0_d, c0_d, h1i_d, c1i_d, h2i_d, c2i_d = [], [], [], [], [], [], []
    for k in range(NCHUNK):
        CB = CBS[k]
        xt_d.append(nc.dram_tensor(f"xt{k}", [2, nblk * 64 * CB], F16,
                                   kind="ExternalInput"))
        s0_d.append(nc.dram_tensor(f"s0{k}", [KW, CB], F16,
                                   kind="ExternalInput"))
        c0_d.append(nc.dram_tensor(f"c0{k}", [48, CB],
                                   mybir.dt.from_np(np.dtype(NPC)),
                                   kind="ExternalInput"))
        h1i_d.append(nc.dram_tensor(f"h1i{k}", [16, CB], F16,
                                    kind="ExternalInput"))
        c1i_d.append(nc.dram_tensor(f"c1i{k}", [16, CB],
                                    mybir.dt.from_np(np.dtype(NPC)),
                                    kind="ExternalInput"))
        h2i_d.append(nc.dram_tensor(f"h2i{k}", [16, CB], F16,
                                    kind="ExternalInput"))
        c2i_d.append(nc.dram_tensor(f"c2i{k}", [16, CB],
                                    mybir.dt.from_np(np.dtype(NPC)),
                                    kind="ExternalInput"))
    y_d = nc.dram_tensor("y", [1, BL], F32, kind="ExternalOutput")

    SIG = mybir.ActivationFunctionType.Sigmoid
    TANH = mybir.ActivationFunctionType.Tanh
    ADD = mybir.AluOpType.add
    MULT = mybir.AluOpType.mult

    with TileContext(nc) as tc:
        wt = nc.alloc_sbuf_tensor("wt_sb", [KW, 256], F16)
        wfc = nc.alloc_sbuf_tensor("wfc_sb", [17, 1], F16)
        hf = nc.alloc_sbuf_tensor("hf_sb", [17, BL], F16)
        ys = nc.alloc_sbuf_tensor("ys_sb", [1, BL], F32)
        # x staging: two 64-step blocks per chunk, rows 49:51 so the
        # per-step DVE copy into S[49:51] is a zero-shift partition access
        xtb, S, C = [], [], []
        for k in range(NCHUNK):
            CB = CBS[k]
            xtb.append([nc.alloc_sbuf_tensor(f"xtb{k}_{j}", [KW, 64 * CB], F16)
                        for j in range(2)])
            S.append(nc.alloc_sbuf_tensor(f"S_sb{k}", [KW, CB], F16))
            C.append(nc.alloc_sbuf_tensor(f"C_sb{k}", [112, CB], CDT))

        def xblk(k, blk):
            """Prefetch 64-step x block `blk` into its staging buffer."""
            CB = CBS[k]
            n0 = blk * 64 * CB
            nc.sync.dma_start(xtb[k][blk % 2][X0:KW, :],
                              xt_d[k].ap()[0:2, n0:n0 + 64 * CB])

        nc.sync.dma_start(wt[:, :], wt_d.ap())
        nc.sync.dma_start(wfc[:, :], wfc_d.ap())
        for k in range(NCHUNK):
            nc.sync.dma_start(S[k][:, :], s0_d[k].ap())
            nc.sync.dma_start(C[k][64:112, :], c0_d[k].ap())
            xblk(k, 0)
            if nblk > 1:
                xblk(k, 1)

        psum_bufs = 1 if sum(2 * cb * 4 * 2 for cb in CBS) > 16384 else 2
        fw_eng = nc.gpsimd if _FW_POOL else nc.vector
        with tc.tile_pool(name="ps", bufs=psum_bufs, space="PSUM") as pps, \
             tc.tile_pool(name="sb", bufs=4) as psb:
            # Emission order = scheduler priority (tie-break among ready
            # ops). Interleave the two chunks at op granularity so chunk b's
            # front fills chunk a's C'->tanh latency gap.
            live = [None] * NCHUNK  # per-chunk (G, HU, FW)

            # Optional static phase-pinning: LSTM_LAM pins a steady-state
            # period (ns); each op class gets a per-macro lower-bound
            # timestamp so the greedy scheduler follows a fixed software
            # pipeline instead of making myopic choices. 0 = off.
            LAM = float(os.environ.get("LSTM_LAM", "0"))
            # op-class offsets within a chunk's cycle (ns from MM issue)
            PH_MM, PH_SIG, PH_MID, PH_TANH, PH_H = 0.0, 293.0, 880.0, 1540.0, 1990.0

            from contextlib import nullcontext

            def pin(w, off):
                if LAM <= 0 or w is None:
                    return nullcontext()
                return tc.tile_wait_until((w + off) * 1e-6)

            def front(k, m, w=None):
                CB = CBS[k]
                P = pps.tile([128, 2, CB], F32, tag=f"P{k}")
                G = psb.tile([128, 2, CB], F16, tag=f"G{k}")
                HU = psb.tile([48, CB], F16, tag=f"HU{k}")
                FW = psb.tile([48, CB], CDT, tag=f"FW{k}")
                with pin(w, PH_MM):
                    # K=66 contraction: h rows, bias row, zero pad, x rows.
                    nc.tensor.matmul(P[0:128, 0, 0:CB], wt[0:KW, 0:128],
                                     S[k][0:KW, :], start=True, stop=True)
                    nc.tensor.matmul(P[0:128, 1, 0:CB], wt[0:KW, 128:256],
                                     S[k][0:KW, :], start=True, stop=True)
                with pin(w, PH_SIG):
                    # One sigmoid across both banks: i,f (bank0) + 2g,o (b1)
                    nc.scalar.activation(G[0:128, 0:2, 0:CB],
                                         P[0:128, 0:2, 0:CB], SIG)
                live[k] = (G, HU, FW)

            _MID_HU_FIRST = os.environ.get("LSTM_HU_FIRST", "1") == "1"

            def mid(k, w=None):
                CB = CBS[k]
                G, HU, FW = live[k]
                with pin(w, PH_MID):
                    # HU first: FW then fills the scheduler's 117ns
                    # same-engine readiness gap before C' instead of an
                    # unrelated op wedging in and delaying tanh.
                    if _MID_HU_FIRST:
                        nc.vector.scalar_tensor_tensor(
                            HU[0:48, :], G[0:48, 1, 0:CB], -0.5,
                            G[0:48, 0, 0:CB], ADD, MULT)
                        fw_eng.tensor_mul(FW[0:48, :], G[64:112, 0, 0:CB],
                                          C[k][64:112, :])
                    else:
                        fw_eng.tensor_mul(FW[0:48, :], G[64:112, 0, 0:CB],
                                          C[k][64:112, :])
                        nc.vector.scalar_tensor_tensor(
                            HU[0:48, :], G[0:48, 1, 0:CB], -0.5,
                            G[0:48, 0, 0:CB], ADD, MULT)

            def cprime(k, w=None):
                G, HU, FW = live[k]
                TC = psb.tile([112, CBS[k]], F16, tag=f"TC{k}")
                with pin(w, PH_MID):
                    # c_half' = hu + fw   (plain TT add -> 2x mode)
                    nc.vector.tensor_add(C[k][64:112, :], HU[0:48, :],
                                         FW[0:48, :])
                with pin(w, PH_TANH):
                    # tc = tanh(2 * c_half') = tanh(c')
                    nc.scalar.activation(TC[64:112, :], C[k][64:112, :], TANH,
                                         scale=2.0)
                live[k] = (G, TC)

            def hout(k, s, w=None):
                CB = CBS[k]
                G, TC = live[k]
                with pin(w, PH_H):
                    # h0,h1,h2 = o * tc in one op
                    nc.vector.tensor_mul(S[k][0:48, :], G[64:112, 1, 0:CB],
                                         TC[64:112, :])
                    # delayed init: overwrite wavefront-startup pollution
                    if s == 0:
                        nc.sync.dma_start(S[k][16:32, :], h1i_d[k].ap())
                        nc.sync.dma_start(C[k][80:96, :], c1i_d[k].ap())
                    elif s == 1:
                        nc.sync.dma_start(S[k][32:48, :], h2i_d[k].ap())
                        nc.sync.dma_start(C[k][96:112, :], c2i_d[k].ap())

            xc_eng = nc.gpsimd if os.environ.get("LSTM_XC_POOL", "1") == "1" \
                else nc.vector

            def xstage(k, s):
                # stage next x: copy from the current 64-step staging block
                # (zero partition shift, base 64 -> legal on Pool too).
                # Pool keeps it off the DVE queue entirely; emitted at the
                # END of the macro's stream so its priority ranks below
                # every cycle-critical op. Correctness is unaffected: Tile
                # orders it after this step's MM reads of S (WAR) and
                # before the next step's (RAW).
                CB = CBS[k]
                if s + 1 < _t_run():
                    nb_, nu = divmod(s + 1, 64)
                    xc_eng.tensor_copy(
                        S[k][X0:KW, :],
                        xtb[k][nb_ % 2][X0:KW, nu * CB:(nu + 1) * CB])
                    # one step into block nb_, its predecessor buffer is
                    # free: prefetch block nb_+1 into it
                    if nu == 1 and nb_ + 1 < nblk:
                        xblk(k, nb_ + 1)

            def wbase(m, k):
                if LAM <= 0:
                    return None
                return m * LAM + k * LAM / NCHUNK

            for m in range(NT):
                if NCHUNK == 2:
                    front(0, m, wbase(m, 0))
                    front(1, m, wbase(m, 1))
                    mid(0, wbase(m, 0))
                    cprime(0, wbase(m, 0))
                    mid(1, wbase(m, 1))
                    hout(0, m, wbase(m, 0))
                    cprime(1, wbase(m, 1))
                    hout(1, m, wbase(m, 1))
                else:
                    for k in range(NCHUNK):
                        front(k, m, wbase(m, k))
                    mid(0, wbase(m, 0))
                    cprime(0, wbase(m, 0))
                    for k in range(1, NCHUNK):
                        mid(k, wbase(m, k))
                        hout(k - 1, m, wbase(m, k - 1))
                        cprime(k, wbase(m, k))
                    hout(NCHUNK - 1, m, wbase(m, NCHUNK - 1))
                for k in range(NCHUNK):
                    xstage(k, m)

        # final fc: y = h2 @ W_fc.T + b_fc
        with tc.tile_pool(name="pf", bufs=1, space="PSUM") as ppf:
            nc.vector.memset(hf[0:17, :], 1.0)
            for k in range(NCHUNK):
                nc.vector.tensor_copy(hf[0:16, OFFS[k]:OFFS[k] + CBS[k]],
                                      S[k][32:48, :])
            PF = ppf.tile([1, BL], F32, tag="PF")
            nc.tensor.matmul(PF[0:1, :], wfc[0:17, 0:1], hf[0:17, :],
                             start=True, stop=True)
            nc.scalar.copy(ys[0:1, :], PF[0:1, :])
            nc.sync.dma_start(y_d.ap(), ys[0:1, :])

    nc.compile()
    return nc


def prep_chunk_inputs(inputs, core, k):
    NPC = np.float32 if _CDT_ENV == "f32" else NPF16
    CB = CBS[k]
    b0 = core * BL + OFFS[k]
    b1 = b0 + CB
    tr = _t_run()
    nblk = (tr + 63) // 64

    x = np.asarray(inputs["x"])[b0:b1]          # [CB, T, IN]
    h0 = np.asarray(inputs["h0"])[:, b0:b1]     # [L, CB, H]
    c0 = np.asarray(inputs["c0"])[:, b0:b1]

    # xt layout: partition = feature, free = t*CB + b (step-major)
    xt = np.zeros((2, nblk * 64 * CB), np.float32)
    xr = x[:, :tr, :].transpose(2, 1, 0)         # [f, t, b]
    xt[:, :tr * CB] = xr.reshape(2, tr * CB)

    s0 = np.zeros((KW, CB), np.float32)
    s0[0:16] = h0[0].T
    s0[16:32] = h0[1].T
    s0[32:48] = h0[2].T
    s0[48] = 1.0
    s0[X0:X0 + 2] = x[:, 0, :].T

    # cell state is stored as c/2 on-device (C-half trick)
    c0p = 0.5 * np.concatenate([c0[0].T, c0[1].T, c0[2].T], axis=0)  # [48, CB]

    return {
        f"xt{k}": xt.astype(NPF16),
        f"s0{k}": s0.astype(NPF16),
        f"c0{k}": np.ascontiguousarray(c0p).astype(NPC),
        f"h1i{k}": np.ascontiguousarray(h0[1].T).astype(NPF16),
        f"c1i{k}": np.ascontiguousarray(0.5 * c0[1].T).astype(NPC),
        f"h2i{k}": np.ascontiguousarray(h0[2].T).astype(NPF16),
        f"c2i{k}": np.ascontiguousarray(0.5 * c0[2].T).astype(NPC),
    }


_NC_CACHE = {}


def kernel(**inputs):
    key = (_t_run(), _CDT_ENV, _FW_POOL, NCHUNK)
    if key not in _NC_CACHE:
        _NC_CACHE[key] = build_bass()
    nc = _NC_CACHE[key]

    b0v = np.asarray(inputs["b_ih0"]) + np.asarray(inputs["b_hh0"])
    b1v = np.asarray(inputs["b_ih1"]) + np.asarray(inputs["b_hh1"])
    b2v = np.asarray(inputs["b_ih2"]) + np.asarray(inputs["b_hh2"])
    WT = build_weight_block(
        np.asarray(inputs["W_ih0"]), np.asarray(inputs["W_hh0"]), b0v,
        np.asarray(inputs["W_ih1"]), np.asarray(inputs["W_hh1"]), b1v,
        np.asarray(inputs["W_ih2"]), np.asarray(inputs["W_hh2"]), b2v,
    ).astype(NPF16)
    wfc = np.zeros((17, 1), np.float32)
    wfc[0:16, 0] = np.asarray(inputs["W_fc"])[0]
    wfc[16, 0] = np.asarray(inputs["b_fc"])[0]
    wfc = wfc.astype(NPF16)

    in_maps = []
    for core in range(NCORES):
        m = {"wt": WT, "wfc": wfc}
        for k in range(NCHUNK):
            m.update(prep_chunk_inputs(inputs, core, k))
        in_maps.append(m)

    trace = os.environ.get("LSTM_TRACE", "0") == "1"
    res = bass_utils.run_bass_kernel_spmd(nc, in_maps, core_ids=list(range(NCORES)),
                                          trace=trace)
    global _LAST_RESULT
    _LAST_RESULT = res
    out = np.concatenate([res.results[c]["y"][0] for c in range(NCORES)])
    return out.reshape(B, 1).astype(np.float32)


_LAST_RESULT = None


if __name__ == "__main__":
    import reference
    inputs = reference.setup_inputs()
    y = kernel(**{k: np.asarray(v) for k, v in inputs.items()})
    print("kernel out", y.shape, y[:4, 0])


# revision 28
# speedup vs baseline: 2.1324x; 2.1324x over previous
"""Trainium2 Bass kernel for a 3-layer LSTM (B=4096, T=1024, IN=2, H=16) + final FC.

Per core (batch-sharded 8 ways, B_local=512), wavefront over layers:
macro-step s computes L0@t=s, L1@t=s-1, L2@t=s-2. The batch is further split
into 2 staggered chunks of 256 so two independent recurrence chains can
interleave across engines (hides the per-step dependency-chain latency).

v2 changes vs baseline (4.29ms):
  - x is read DIRECTLY by the PE as a second accumulating K=2 matmul per
    PSUM bank (tile_position=(0,0)), killing the per-step SBUF->SBUF x-stage
    DMA that sat on the recurrence critical path (SP seq ~1.4us per step).
  - ONE merged sigmoid per chunk-step over both PSUM banks (FD=512) instead
    of two FD=256 ops (saves one ACT fixed cost per chunk-step).
  - Cell state is stored as c/2 ("C-half"): then c'/2 = HU + FW is a plain
    2x-mode tensor_tensor ADD (194ns) instead of a 1x STT (327ns), and
    tanh(c') = tanh(2 * C_half) uses ACT's free scale=2 affine.
  - FW = f*C_half runs on the (otherwise idle) Pool engine to keep DVE
    under the ACT roofline.

Stationary weights WT [51, 256] f16: rows 0:48 = h0,h1,h2 recurrent/inter-
layer weights, row 48 = bias, rows 49:51 = x weights (layer 0 only).
Moving window S [49, CB]: rows 0:48 h, row 48 = ones (bias).
Gate cols per 128-half: first group (i or 2g) at 16*l, second (f or o) at
64+16*l; g's weights+bias prescaled x2 so sigmoid(2g) = (tanh(g)+1)/2.

Per chunk-step:
  MM_Ah (K=49, start) + MM_Ax (K=2, stop)  -> PSUM bank A rows: i(0:48) f(64:112)
  MM_Bh + MM_Bx                            -> PSUM bank B rows: 2g, o
  SIG  G[128,2,CB] = sigmoid(P)            ACT, one op FD=2*CB
  FW   = G_f * C_half                      Pool tensor_mul
  HU   = (G_2g - 0.5) * G_i                DVE STT  [= i*tanh(g)/2]
  C_half' = HU + FW                        DVE tensor_add (2x mode)
  TC   = tanh(2 * C_half')                 ACT scale=2
  S[0:48] = G_o * TC  (h0,h1,h2 at once)   DVE tensor_mul
"""

import os
import sys

sys.path.insert(0, "/opt/trn_rl_repo")

import numpy as np

import concourse.bacc as bacc
import concourse.mybir as mybir
from concourse.tile import TileContext
from concourse import bass_utils

B, T, IN, H, L = 4096, 1024, 2, 16, 3
NCORES = 8
BL = B // NCORES          # 512
NCHUNK = int(os.environ.get("LSTM_NCHUNK", "3"))
if BL % NCHUNK == 0:
    CBS = [BL // NCHUNK] * NCHUNK
else:
    # uneven chunks (e.g. NCHUNK=3 -> 172,172,168); keep sizes even for
    # DVE 2x mode
    base = (BL // NCHUNK) // 2 * 2
    CBS = [base + 2] * ((BL - base * NCHUNK) // 2)
    CBS += [base] * (NCHUNK - len(CBS))
    assert sum(CBS) == BL and all(c % 2 == 0 for c in CBS), CBS
OFFS = [sum(CBS[:k]) for k in range(NCHUNK)]
F32 = mybir.dt.float32
F16 = mybir.dt.float16
NPF16 = np.float16

# PyTorch gate rows in W_ih*/W_hh*: i, f, g, o
PT_I, PT_F, PT_G, PT_O = slice(0, 16), slice(16, 32), slice(32, 48), slice(48, 64)

_STEPS_ENV = int(os.environ.get("LSTM_STEPS", "0"))
_FW_POOL = os.environ.get("LSTM_FW_POOL", "0") == "1"
_CDT_ENV = os.environ.get("LSTM_CDT", "f16")  # cell-state dtype: f32 | f16


def _t_run():
    return _STEPS_ENV if _STEPS_ENV > 0 else T


KH = 49  # h rows + bias row
X0 = 64  # x rows base: must be 32-aligned for engine (DVE) partition access
KW = 66  # total stationary/moving contraction rows (h + bias + pad + 2 x)


def build_weight_block(W_ih0, W_hh0, b0, W_ih1, W_hh1, b1, W_ih2, W_hh2, b2):
    """WT [51, 256] f32. Cols 0:128 = bank A (i,f), 128:256 = bank B (2g, o).

    K rows: 0:16 h0, 16:32 h1, 32:48 h2, 48 one(bias), 49:51 x.
    Col layout within each bank: layer-l gate block at 16*l : 16*l+16 for
    the first gate group (i or g), 64+16*l : 64+16*l+16 for the second (f or o).
    """
    WT = np.zeros((KW, 256), np.float32)
    layers = [
        # (in_rows, rec_rows, W_ih, W_hh, b)
        (slice(X0, X0 + 2), slice(0, 16), W_ih0, W_hh0, b0),
        (slice(0, 16), slice(16, 32), W_ih1, W_hh1, b1),
        (slice(16, 32), slice(32, 48), W_ih2, W_hh2, b2),
    ]
    for half, (pt0, pt1) in ((0, (PT_I, PT_F)), (128, (PT_G, PT_O))):
        for l, (ir, rr, Wih, Whh, b) in enumerate(layers):
            for pt, cbase, sc in ((pt0, half + 16 * l, 2.0 if half else 1.0),
                                  (pt1, half + 64 + 16 * l, 1.0)):
                WT[ir, cbase:cbase + 16] = sc * Wih[pt].T
                WT[rr, cbase:cbase + 16] = sc * Whh[pt].T
                WT[48, cbase:cbase + 16] = sc * b[pt]
    return WT


def build_bass():
    CDT = F32 if _CDT_ENV == "f32" else F16
    NPC = np.float32 if _CDT_ENV == "f32" else NPF16
    nc = bacc.Bacc("TRN2", target_bir_lowering=False, debug=False,
                   num_devices=NCORES)
    NT = _t_run() + 2
    nblk = (_t_run() + 63) // 64

    wt_d = nc.dram_tensor("wt", [KW, 256], F16, kind="ExternalInput")
    wfc_d = nc.dram_tensor("wfc", [17, 1], F16, kind="ExternalInput")
    xt_d, s0_d, c0_d, h1i_d, c1i_d, h2i_d, c2i_d = [], [], [], [], [], [], []
    for k in range(NCHUNK):
        CB = CBS[k]
        xt_d.append(nc.dram_tensor(f"xt{k}", [2, nblk * 64 * CB], F16,
                                   kind="ExternalInput"))
        s0_d.append(nc.dram_tensor(f"s0{k}", [KW, CB], F16,
                                   kind="ExternalInput"))
        c0_d.append(nc.dram_tensor(f"c0{k}", [48, CB],
                                   mybir.dt.from_np(np.dtype(NPC)),
                                   kind="ExternalInput"))
        h1i_d.append(nc.dram_tensor(f"h1i{k}", [16, CB], F16,
                                    kind="ExternalInput"))
        c1i_d.append(nc.dram_tensor(f"c1i{k}", [16, CB],
                                    mybir.dt.from_np(np.dtype(NPC)),
                                    kind="ExternalInput"))
        h2i_d.append(nc.dram_tensor(f"h2i{k}", [16, CB], F16,
                                    kind="ExternalInput"))
        c2i_d.append(nc.dram_tensor(f"c2i{k}", [16, CB],
                                    mybir.dt.from_np(np.dtype(NPC)),
                                    kind="ExternalInput"))
    y_d = nc.dram_tensor("y", [1, BL], F32, kind="ExternalOutput")

    SIG = mybir.ActivationFunctionType.Sigmoid
    TANH = mybir.ActivationFunctionType.Tanh
    ADD = mybir.AluOpType.add
    MULT = mybir.AluOpType.mult

    with TileContext(nc) as tc:
        wt = nc.alloc_sbuf_tensor("wt_sb", [KW, 256], F16)
        wfc = nc.alloc_sbuf_tensor("wfc_sb", [17, 1], F16)
        hf = nc.alloc_sbuf_tensor("hf_sb", [17, BL], F16)
        ys = nc.alloc_sbuf_tensor("ys_sb", [1, BL], F32)
        # x staging: two 64-step blocks per chunk, rows 49:51 so the
        # per-step DVE copy into S[49:51] is a zero-shift partition access
        xtb, S, C = [], [], []
        for k in range(NCHUNK):
            CB = CBS[k]
            xtb.append([nc.alloc_sbuf_tensor(f"xtb{k}_{j}", [KW, 64 * CB], F16)
                        for j in range(2)])
            S.append(nc.alloc_sbuf_tensor(f"S_sb{k}", [KW, CB], F16))
            C.append(nc.alloc_sbuf_tensor(f"C_sb{k}", [112, CB], CDT))

        def xblk(k, blk):
            """Prefetch 64-step x block `blk` into its staging buffer."""
            CB = CBS[k]
            n0 = blk * 64 * CB
            nc.sync.dma_start(xtb[k][blk % 2][X0:KW, :],
                              xt_d[k].ap()[0:2, n0:n0 + 64 * CB])

        nc.sync.dma_start(wt[:, :], wt_d.ap())
        nc.sync.dma_start(wfc[:, :], wfc_d.ap())
        for k in range(NCHUNK):
            nc.sync.dma_start(S[k][:, :], s0_d[k].ap())
            nc.sync.dma_start(C[k][64:112, :], c0_d[k].ap())
            xblk(k, 0)
            if nblk > 1:
                xblk(k, 1)

        psum_bufs = 1 if sum(2 * cb * 4 * 2 for cb in CBS) > 16384 else 2
        fw_eng = nc.gpsimd if _FW_POOL else nc.vector
        with tc.tile_pool(name="ps", bufs=psum_bufs, space="PSUM") as pps, \
             tc.tile_pool(name="sb", bufs=4) as psb:
            # Emission order = scheduler priority (tie-break among ready
            # ops). Interleave the two chunks at op granularity so chunk b's
            # front fills chunk a's C'->tanh latency gap.
            live = [None] * NCHUNK  # per-chunk (G, HU, FW)

            # Optional static phase-pinning: LSTM_LAM pins a steady-state
            # period (ns); each op class gets a per-macro lower-bound
            # timestamp so the greedy scheduler follows a fixed software
            # pipeline instead of making myopic choices. 0 = off.
            LAM = float(os.environ.get("LSTM_LAM", "0"))
            # op-class offsets within a chunk's cycle (ns from MM issue)
            PH_MM, PH_SIG, PH_MID, PH_TANH, PH_H = 0.0, 293.0, 880.0, 1540.0, 1990.0

            from contextlib import nullcontext

            def pin(w, off):
                if LAM <= 0 or w is None:
                    return nullcontext()
                return tc.tile_wait_until((w + off) * 1e-6)

            def front(k, m, w=None):
                CB = CBS[k]
                P = pps.tile([128, 2, CB], F32, tag=f"P{k}")
                G = psb.tile([128, 2, CB], F16, tag=f"G{k}")
                HU = psb.tile([48, CB], F16, tag=f"HU{k}")
                FW = psb.tile([48, CB], CDT, tag=f"FW{k}")
                with pin(w, PH_MM):
                    # K=66 contraction: h rows, bias row, zero pad, x rows.
                    nc.tensor.matmul(P[0:128, 0, 0:CB], wt[0:KW, 0:128],
                                     S[k][0:KW, :], start=True, stop=True)
                    nc.tensor.matmul(P[0:128, 1, 0:CB], wt[0:KW, 128:256],
                                     S[k][0:KW, :], start=True, stop=True)
                with pin(w, PH_SIG):
                    # One sigmoid across both banks: i,f (bank0) + 2g,o (b1)
                    nc.scalar.activation(G[0:128, 0:2, 0:CB],
                                         P[0:128, 0:2, 0:CB], SIG)
                live[k] = (G, HU, FW)

            _MID_HU_FIRST = os.environ.get("LSTM_HU_FIRST", "1") == "1"

            def mid(k, w=None):
                CB = CBS[k]
                G, HU, FW = live[k]
                with pin(w, PH_MID):
                    # HU first: FW then fills the scheduler's 117ns
                    # same-engine readiness gap before C' instead of an
                    # unrelated op wedging in and delaying tanh.
                    if _MID_HU_FIRST:
                        nc.vector.scalar_tensor_tensor(
                            HU[0:48, :], G[0:48, 1, 0:CB], -0.5,
                            G[0:48, 0, 0:CB], ADD, MULT)
                        fw_eng.tensor_mul(FW[0:48, :], G[64:112, 0, 0:CB],
                                          C[k][64:112, :])
                    else:
                        fw_eng.tensor_mul(FW[0:48, :], G[64:112, 0, 0:CB],
                                          C[k][64:112, :])
                        nc.vector.scalar_tensor_tensor(
                            HU[0:48, :], G[0:48, 1, 0:CB], -0.5,
                            G[0:48, 0, 0:CB], ADD, MULT)

            def cprime(k, w=None):
                G, HU, FW = live[k]
                TC = psb.tile([112, CBS[k]], F16, tag=f"TC{k}")
                with pin(w, PH_MID):
                    # c_half' = hu + fw   (plain TT add -> 2x mode)
                    nc.vector.tensor_add(C[k][64:112, :], HU[0:48, :],
                                         FW[0:48, :])
                with pin(w, PH_TANH):
                    # tc = tanh(2 * c_half') = tanh(c')
                    nc.scalar.activation(TC[64:112, :], C[k][64:112, :], TANH,
                                         scale=2.0)
                live[k] = (G, TC)

            def hout(k, s, w=None):
                CB = CBS[k]
                G, TC = live[k]
                with pin(w, PH_H):
                    # h0,h1,h2 = o * tc in one op
                    nc.vector.tensor_mul(S[k][0:48, :], G[64:112, 1, 0:CB],
                                         TC[64:112, :])
                    # delayed init: overwrite wavefront-startup pollution
                    if s == 0:
                        nc.sync.dma_start(S[k][16:32, :], h1i_d[k].ap())
                        nc.sync.dma_start(C[k][80:96, :], c1i_d[k].ap())
                    elif s == 1:
                        nc.sync.dma_start(S[k][32:48, :], h2i_d[k].ap())
                        nc.sync.dma_start(C[k][96:112, :], c2i_d[k].ap())

            xc_eng = nc.gpsimd if os.environ.get("LSTM_XC_POOL", "0") == "1" \
                else nc.vector

            def xstage(k, s):
                # stage next x: copy from the current 64-step staging block
                # (zero partition shift, base 64 -> legal on Pool too).
                # Pool keeps it off the DVE queue entirely; emitted at the
                # END of the macro's stream so its priority ranks below
                # every cycle-critical op. Correctness is unaffected: Tile
                # orders it after this step's MM reads of S (WAR) and
                # before the next step's (RAW).
                CB = CBS[k]
                if s + 1 < _t_run():
                    nb_, nu = divmod(s + 1, 64)
                    xc_eng.tensor_copy(
                        S[k][X0:KW, :],
                        xtb[k][nb_ % 2][X0:KW, nu * CB:(nu + 1) * CB])
                    # one step into block nb_, its predecessor buffer is
                    # free: prefetch block nb_+1 into it
                    if nu == 1 and nb_ + 1 < nblk:
                        xblk(k, nb_ + 1)

            def wbase(m, k):
                if LAM <= 0:
                    return None
                return m * LAM + k * LAM / NCHUNK

            for m in range(NT):
                if NCHUNK == 2:
                    front(0, m, wbase(m, 0))
                    front(1, m, wbase(m, 1))
                    mid(0, wbase(m, 0))
                    cprime(0, wbase(m, 0))
                    mid(1, wbase(m, 1))
                    hout(0, m, wbase(m, 0))
                    cprime(1, wbase(m, 1))
                    hout(1, m, wbase(m, 1))
                else:
                    for k in range(NCHUNK):
                        front(k, m, wbase(m, k))
                    mid(0, wbase(m, 0))
                    cprime(0, wbase(m, 0))
                    for k in range(1, NCHUNK):
                        mid(k, wbase(m, k))
                        hout(k - 1, m, wbase(m, k - 1))
                        cprime(k, wbase(m, k))
                    hout(NCHUNK - 1, m, wbase(m, NCHUNK - 1))
                for k in range(NCHUNK):
                    xstage(k, m)

        # final fc: y = h2 @ W_fc.T + b_fc
        with tc.tile_pool(name="pf", bufs=1, space="PSUM") as ppf:
            nc.vector.memset(hf[0:17, :], 1.0)
            for k in range(NCHUNK):
                nc.vector.tensor_copy(hf[0:16, OFFS[k]:OFFS[k] + CBS[k]],
                                      S[k][32:48, :])
            PF = ppf.tile([1, BL], F32, tag="PF")
            nc.tensor.matmul(PF[0:1, :], wfc[0:17, 0:1], hf[0:17, :],
                             start=True, stop=True)
            nc.scalar.copy(ys[0:1, :], PF[0:1, :])
            nc.sync.dma_start(y_d.ap(), ys[0:1, :])

    nc.compile()
    return nc


def prep_chunk_inputs(inputs, core, k):
    NPC = np.float32 if _CDT_ENV == "f32" else NPF16
    CB = CBS[k]
    b0 = core * BL + OFFS[k]
    b1 = b0 + CB
    tr = _t_run()
    nblk = (tr + 63) // 64

    x = np.asarray(inputs["x"])[b0:b1]          # [CB, T, IN]
    h0 = np.asarray(inputs["h0"])[:, b0:b1]     # [L, CB, H]
    c0 = np.asarray(inputs["c0"])[:, b0:b1]

    # xt layout: partition = feature, free = t*CB + b (step-major)
    xt = np.zeros((2, nblk * 64 * CB), np.float32)
    xr = x[:, :tr, :].transpose(2, 1, 0)         # [f, t, b]
    xt[:, :tr * CB] = xr.reshape(2, tr * CB)

    s0 = np.zeros((KW, CB), np.float32)
    s0[0:16] = h0[0].T
    s0[16:32] = h0[1].T
    s0[32:48] = h0[2].T
    s0[48] = 1.0
    s0[X0:X0 + 2] = x[:, 0, :].T

    # cell state is stored as c/2 on-device (C-half trick)
    c0p = 0.5 * np.concatenate([c0[0].T, c0[1].T, c0[2].T], axis=0)  # [48, CB]

    return {
        f"xt{k}": xt.astype(NPF16),
        f"s0{k}": s0.astype(NPF16),
        f"c0{k}": np.ascontiguousarray(c0p).astype(NPC),
        f"h1i{k}": np.ascontiguousarray(h0[1].T).astype(NPF16),
        f"c1i{k}": np.ascontiguousarray(0.5 * c0[1].T).astype(NPC),
        f"h2i{k}": np.ascontiguousarray(h0[2].T).astype(NPF16),
        f"c2i{k}": np.ascontiguousarray(0.5 * c0[2].T).astype(NPC),
    }


_NC_CACHE = {}


def kernel(**inputs):
    key = (_t_run(), _CDT_ENV, _FW_POOL, NCHUNK)
    if key not in _NC_CACHE:
        _NC_CACHE[key] = build_bass()
    nc = _NC_CACHE[key]

    b0v = np.asarray(inputs["b_ih0"]) + np.asarray(inputs["b_hh0"])
    b1v = np.asarray(inputs["b_ih1"]) + np.asarray(inputs["b_hh1"])
    b2v = np.asarray(inputs["b_ih2"]) + np.asarray(inputs["b_hh2"])
    WT = build_weight_block(
        np.asarray(inputs["W_ih0"]), np.asarray(inputs["W_hh0"]), b0v,
        np.asarray(inputs["W_ih1"]), np.asarray(inputs["W_hh1"]), b1v,
        np.asarray(inputs["W_ih2"]), np.asarray(inputs["W_hh2"]), b2v,
    ).astype(NPF16)
    wfc = np.zeros((17, 1), np.float32)
    wfc[0:16, 0] = np.asarray(inputs["W_fc"])[0]
    wfc[16, 0] = np.asarray(inputs["b_fc"])[0]
    wfc = wfc.astype(NPF16)

    in_maps = []
    for core in range(NCORES):
        m = {"wt": WT, "wfc": wfc}
        for k in range(NCHUNK):
            m.update(prep_chunk_inputs(inputs, core, k))
        in_maps.append(m)

    trace = os.environ.get("LSTM_TRACE", "0") == "1"
    res = bass_utils.run_bass_kernel_spmd(nc, in_maps, core_ids=list(range(NCORES)),
                                          trace=trace)
    global _LAST_RESULT
    _LAST_RESULT = res
    out = np.concatenate([res.results[c]["y"][0] for c in range(NCORES)])
    return out.reshape(B, 1).astype(np.float32)


_LAST_RESULT = None


if __name__ == "__main__":
    import reference
    inputs = reference.setup_inputs()
    y = kernel(**{k: np.asarray(v) for k, v in inputs.items()})
    print("kernel out", y.shape, y[:4, 0])


# revision 29
# speedup vs baseline: 6.8719x; 3.2226x over previous
"""Trainium2 Bass kernel for a 3-layer LSTM (B=4096, T=1024, IN=2, H=16) + final FC.

Per core (batch-sharded 8 ways, B_local=512), wavefront over layers:
macro-step s computes L0@t=s, L1@t=s-1, L2@t=s-2. The batch is further split
into 2 staggered chunks of 256 so two independent recurrence chains can
interleave across engines (hides the per-step dependency-chain latency).

v2 changes vs baseline (4.29ms):
  - x is read DIRECTLY by the PE as a second accumulating K=2 matmul per
    PSUM bank (tile_position=(0,0)), killing the per-step SBUF->SBUF x-stage
    DMA that sat on the recurrence critical path (SP seq ~1.4us per step).
  - ONE merged sigmoid per chunk-step over both PSUM banks (FD=512) instead
    of two FD=256 ops (saves one ACT fixed cost per chunk-step).
  - Cell state is stored as c/2 ("C-half"): then c'/2 = HU + FW is a plain
    2x-mode tensor_tensor ADD (194ns) instead of a 1x STT (327ns), and
    tanh(c') = tanh(2 * C_half) uses ACT's free scale=2 affine.
  - FW = f*C_half runs on the (otherwise idle) Pool engine to keep DVE
    under the ACT roofline.

Stationary weights WT [51, 256] f16: rows 0:48 = h0,h1,h2 recurrent/inter-
layer weights, row 48 = bias, rows 49:51 = x weights (layer 0 only).
Moving window S [49, CB]: rows 0:48 h, row 48 = ones (bias).
Gate cols per 128-half: first group (i or 2g) at 16*l, second (f or o) at
64+16*l; g's weights+bias prescaled x2 so sigmoid(2g) = (tanh(g)+1)/2.

Per chunk-step:
  MM_Ah (K=49, start) + MM_Ax (K=2, stop)  -> PSUM bank A rows: i(0:48) f(64:112)
  MM_Bh + MM_Bx                            -> PSUM bank B rows: 2g, o
  SIG  G[128,2,CB] = sigmoid(P)            ACT, one op FD=2*CB
  FW   = G_f * C_half                      Pool tensor_mul
  HU   = (G_2g - 0.5) * G_i                DVE STT  [= i*tanh(g)/2]
  C_half' = HU + FW                        DVE tensor_add (2x mode)
  TC   = tanh(2 * C_half')                 ACT scale=2
  S[0:48] = G_o * TC  (h0,h1,h2 at once)   DVE tensor_mul
"""

import os
import sys

sys.path.insert(0, "/opt/trn_rl_repo")

import numpy as np

import concourse.bacc as bacc
import concourse.mybir as mybir
from concourse.tile import TileContext
from concourse import bass_utils

B, T, IN, H, L = 4096, 1024, 2, 16, 3
NCORES = 8
BL = B // NCORES          # 512
NCHUNK = int(os.environ.get("LSTM_NCHUNK", "3"))
if BL % NCHUNK == 0:
    CBS = [BL // NCHUNK] * NCHUNK
else:
    # uneven chunks (e.g. NCHUNK=3 -> 172,172,168); keep sizes even for
    # DVE 2x mode
    base = (BL // NCHUNK) // 2 * 2
    CBS = [base + 2] * ((BL - base * NCHUNK) // 2)
    CBS += [base] * (NCHUNK - len(CBS))
    assert sum(CBS) == BL and all(c % 2 == 0 for c in CBS), CBS
OFFS = [sum(CBS[:k]) for k in range(NCHUNK)]
F32 = mybir.dt.float32
F16 = mybir.dt.float16
NPF16 = np.float16

# PyTorch gate rows in W_ih*/W_hh*: i, f, g, o
PT_I, PT_F, PT_G, PT_O = slice(0, 16), slice(16, 32), slice(32, 48), slice(48, 64)

_STEPS_ENV = int(os.environ.get("LSTM_STEPS", "0"))
_FW_POOL = os.environ.get("LSTM_FW_POOL", "1") == "1"
_CDT_ENV = os.environ.get("LSTM_CDT", "f16")  # cell-state dtype: f32 | f16


def _t_run():
    return _STEPS_ENV if _STEPS_ENV > 0 else T


KH = 49  # h rows + bias row
X0 = 64  # x rows base: must be 32-aligned for engine (DVE) partition access
KW = 66  # total stationary/moving contraction rows (h + bias + pad + 2 x)


def build_weight_block(W_ih0, W_hh0, b0, W_ih1, W_hh1, b1, W_ih2, W_hh2, b2):
    """WT [51, 256] f32. Cols 0:128 = bank A (i,f), 128:256 = bank B (2g, o).

    K rows: 0:16 h0, 16:32 h1, 32:48 h2, 48 one(bias), 49:51 x.
    Col layout within each bank: layer-l gate block at 16*l : 16*l+16 for
    the first gate group (i or g), 64+16*l : 64+16*l+16 for the second (f or o).
    """
    WT = np.zeros((KW, 256), np.float32)
    layers = [
        # (in_rows, rec_rows, W_ih, W_hh, b)
        (slice(X0, X0 + 2), slice(0, 16), W_ih0, W_hh0, b0),
        (slice(0, 16), slice(16, 32), W_ih1, W_hh1, b1),
        (slice(16, 32), slice(32, 48), W_ih2, W_hh2, b2),
    ]
    for half, (pt0, pt1) in ((0, (PT_I, PT_F)), (128, (PT_G, PT_O))):
        for l, (ir, rr, Wih, Whh, b) in enumerate(layers):
            for pt, cbase, sc in ((pt0, half + 16 * l, 2.0 if half else 1.0),
                                  (pt1, half + 64 + 16 * l, 1.0)):
                WT[ir, cbase:cbase + 16] = sc * Wih[pt].T
                WT[rr, cbase:cbase + 16] = sc * Whh[pt].T
                WT[48, cbase:cbase + 16] = sc * b[pt]
    return WT


def build_bass():
    CDT = F32 if _CDT_ENV == "f32" else F16
    NPC = np.float32 if _CDT_ENV == "f32" else NPF16
    nc = bacc.Bacc("TRN2", target_bir_lowering=False, debug=False,
                   num_devices=NCORES)
    NT = _t_run() + 2
    nblk = (_t_run() + 63) // 64

    wt_d = nc.dram_tensor("wt", [KW, 256], F16, kind="ExternalInput")
    wfc_d = nc.dram_tensor("wfc", [17, 1], F16, kind="ExternalInput")
    xt_d, s0_d, c0_d, h1i_d, c1i_d, h2i_d, c2i_d = [], [], [], [], [], [], []
    for k in range(NCHUNK):
        CB = CBS[k]
        xt_d.append(nc.dram_tensor(f"xt{k}", [2, nblk * 64 * CB], F16,
                                   kind="ExternalInput"))
        s0_d.append(nc.dram_tensor(f"s0{k}", [KW, CB], F16,
                                   kind="ExternalInput"))
        c0_d.append(nc.dram_tensor(f"c0{k}", [48, CB],
                                   mybir.dt.from_np(np.dtype(NPC)),
                                   kind="ExternalInput"))
        h1i_d.append(nc.dram_tensor(f"h1i{k}", [16, CB], F16,
                                    kind="ExternalInput"))
        c1i_d.append(nc.dram_tensor(f"c1i{k}", [16, CB],
                                    mybir.dt.from_np(np.dtype(NPC)),
                                    kind="ExternalInput"))
        h2i_d.append(nc.dram_tensor(f"h2i{k}", [16, CB], F16,
                                    kind="ExternalInput"))
        c2i_d.append(nc.dram_tensor(f"c2i{k}", [16, CB],
                                    mybir.dt.from_np(np.dtype(NPC)),
                                    kind="ExternalInput"))
    y_d = nc.dram_tensor("y", [1, BL], F32, kind="ExternalOutput")

    SIG = mybir.ActivationFunctionType.Sigmoid
    TANH = mybir.ActivationFunctionType.Tanh
    ADD = mybir.AluOpType.add
    MULT = mybir.AluOpType.mult

    with TileContext(nc) as tc:
        wt = nc.alloc_sbuf_tensor("wt_sb", [KW, 256], F16)
        wfc = nc.alloc_sbuf_tensor("wfc_sb", [17, 1], F16)
        hf = nc.alloc_sbuf_tensor("hf_sb", [17, BL], F16)
        ys = nc.alloc_sbuf_tensor("ys_sb", [1, BL], F32)
        # x staging: two 64-step blocks per chunk, rows 49:51 so the
        # per-step DVE copy into S[49:51] is a zero-shift partition access
        xtb, S, C = [], [], []
        for k in range(NCHUNK):
            CB = CBS[k]
            xtb.append([nc.alloc_sbuf_tensor(f"xtb{k}_{j}", [KW, 64 * CB], F16)
                        for j in range(2)])
            S.append(nc.alloc_sbuf_tensor(f"S_sb{k}", [KW, CB], F16))
            C.append(nc.alloc_sbuf_tensor(f"C_sb{k}", [112, CB], CDT))

        def xblk(k, blk):
            """Prefetch 64-step x block `blk` into its staging buffer."""
            CB = CBS[k]
            n0 = blk * 64 * CB
            nc.sync.dma_start(xtb[k][blk % 2][X0:KW, :],
                              xt_d[k].ap()[0:2, n0:n0 + 64 * CB])

        nc.sync.dma_start(wt[:, :], wt_d.ap())
        nc.sync.dma_start(wfc[:, :], wfc_d.ap())
        for k in range(NCHUNK):
            nc.sync.dma_start(S[k][:, :], s0_d[k].ap())
            nc.sync.dma_start(C[k][64:112, :], c0_d[k].ap())
            xblk(k, 0)
            if nblk > 1:
                xblk(k, 1)

        psum_bufs = 1 if sum(2 * cb * 4 * 2 for cb in CBS) > 16384 else 2
        fw_eng = nc.gpsimd if _FW_POOL else nc.vector
        with tc.tile_pool(name="ps", bufs=psum_bufs, space="PSUM") as pps, \
             tc.tile_pool(name="sb", bufs=4) as psb:
            # Emission order = scheduler priority (tie-break among ready
            # ops). Interleave the two chunks at op granularity so chunk b's
            # front fills chunk a's C'->tanh latency gap.
            live = [None] * NCHUNK  # per-chunk (G, HU, FW)

            # Optional static phase-pinning: LSTM_LAM pins a steady-state
            # period (ns); each op class gets a per-macro lower-bound
            # timestamp so the greedy scheduler follows a fixed software
            # pipeline instead of making myopic choices. 0 = off.
            LAM = float(os.environ.get("LSTM_LAM", "0"))
            # op-class offsets within a chunk's cycle (ns from MM issue)
            PH_MM, PH_SIG, PH_MID, PH_TANH, PH_H = 0.0, 293.0, 880.0, 1540.0, 1990.0

            from contextlib import nullcontext

            def pin(w, off):
                if LAM <= 0 or w is None:
                    return nullcontext()
                return tc.tile_wait_until((w + off) * 1e-6)

            def front(k, m, w=None):
                CB = CBS[k]
                P = pps.tile([128, 2, CB], F32, tag=f"P{k}")
                G = psb.tile([128, 2, CB], F16, tag=f"G{k}")
                HU = psb.tile([48, CB], F16, tag=f"HU{k}")
                FW = psb.tile([48, CB], CDT, tag=f"FW{k}")
                with pin(w, PH_MM):
                    # K=66 contraction: h rows, bias row, zero pad, x rows.
                    nc.tensor.matmul(P[0:128, 0, 0:CB], wt[0:KW, 0:128],
                                     S[k][0:KW, :], start=True, stop=True)
                    nc.tensor.matmul(P[0:128, 1, 0:CB], wt[0:KW, 128:256],
                                     S[k][0:KW, :], start=True, stop=True)
                with pin(w, PH_SIG):
                    # One sigmoid across both banks: i,f (bank0) + 2g,o (b1)
                    nc.scalar.activation(G[0:128, 0:2, 0:CB],
                                         P[0:128, 0:2, 0:CB], SIG)
                live[k] = (G, HU, FW)

            _MID_HU_FIRST = os.environ.get("LSTM_HU_FIRST", "1") == "1"

            def mid(k, w=None):
                CB = CBS[k]
                G, HU, FW = live[k]
                with pin(w, PH_MID):
                    # HU first: FW then fills the scheduler's 117ns
                    # same-engine readiness gap before C' instead of an
                    # unrelated op wedging in and delaying tanh.
                    if _MID_HU_FIRST:
                        nc.vector.scalar_tensor_tensor(
                            HU[0:48, :], G[0:48, 1, 0:CB], -0.5,
                            G[0:48, 0, 0:CB], ADD, MULT)
                        fw_eng.tensor_mul(FW[0:48, :], G[64:112, 0, 0:CB],
                                          C[k][64:112, :])
                    else:
                        fw_eng.tensor_mul(FW[0:48, :], G[64:112, 0, 0:CB],
                                          C[k][64:112, :])
                        nc.vector.scalar_tensor_tensor(
                            HU[0:48, :], G[0:48, 1, 0:CB], -0.5,
                            G[0:48, 0, 0:CB], ADD, MULT)

            def cprime(k, w=None):
                G, HU, FW = live[k]
                TC = psb.tile([112, CBS[k]], F16, tag=f"TC{k}")
                with pin(w, PH_MID):
                    # c_half' = hu + fw   (plain TT add -> 2x mode)
                    nc.vector.tensor_add(C[k][64:112, :], HU[0:48, :],
                                         FW[0:48, :])
                with pin(w, PH_TANH):
                    # tc = tanh(2 * c_half') = tanh(c')
                    nc.scalar.activation(TC[64:112, :], C[k][64:112, :], TANH,
                                         scale=2.0)
                live[k] = (G, TC)

            def hout(k, s, w=None):
                CB = CBS[k]
                G, TC = live[k]
                with pin(w, PH_H):
                    # h0,h1,h2 = o * tc in one op
                    nc.vector.tensor_mul(S[k][0:48, :], G[64:112, 1, 0:CB],
                                         TC[64:112, :])
                    # delayed init: overwrite wavefront-startup pollution
                    if s == 0:
                        nc.sync.dma_start(S[k][16:32, :], h1i_d[k].ap())
                        nc.sync.dma_start(C[k][80:96, :], c1i_d[k].ap())
                    elif s == 1:
                        nc.sync.dma_start(S[k][32:48, :], h2i_d[k].ap())
                        nc.sync.dma_start(C[k][96:112, :], c2i_d[k].ap())

            xc_eng = nc.gpsimd if os.environ.get("LSTM_XC_POOL", "1") == "1" \
                else nc.vector

            def xstage(k, s):
                # stage next x: copy from the current 64-step staging block
                # (zero partition shift, base 64 -> legal on Pool too).
                # Pool keeps it off the DVE queue entirely; emitted at the
                # END of the macro's stream so its priority ranks below
                # every cycle-critical op. Correctness is unaffected: Tile
                # orders it after this step's MM reads of S (WAR) and
                # before the next step's (RAW).
                CB = CBS[k]
                if s + 1 < _t_run():
                    nb_, nu = divmod(s + 1, 64)
                    xc_eng.tensor_copy(
                        S[k][X0:KW, :],
                        xtb[k][nb_ % 2][X0:KW, nu * CB:(nu + 1) * CB])
                    # one step into block nb_, its predecessor buffer is
                    # free: prefetch block nb_+1 into it
                    if nu == 1 and nb_ + 1 < nblk:
                        xblk(k, nb_ + 1)

            def wbase(m, k):
                if LAM <= 0:
                    return None
                return m * LAM + k * LAM / NCHUNK

            for m in range(NT):
                if NCHUNK == 2:
                    front(0, m, wbase(m, 0))
                    front(1, m, wbase(m, 1))
                    mid(0, wbase(m, 0))
                    cprime(0, wbase(m, 0))
                    mid(1, wbase(m, 1))
                    hout(0, m, wbase(m, 0))
                    cprime(1, wbase(m, 1))
                    hout(1, m, wbase(m, 1))
                else:
                    for k in range(NCHUNK):
                        front(k, m, wbase(m, k))
                    mid(0, wbase(m, 0))
                    cprime(0, wbase(m, 0))
                    for k in range(1, NCHUNK):
                        mid(k, wbase(m, k))
                        hout(k - 1, m, wbase(m, k - 1))
                        cprime(k, wbase(m, k))
                    hout(NCHUNK - 1, m, wbase(m, NCHUNK - 1))
                for k in range(NCHUNK):
                    xstage(k, m)

        # final fc: y = h2 @ W_fc.T + b_fc
        with tc.tile_pool(name="pf", bufs=1, space="PSUM") as ppf:
            nc.vector.memset(hf[0:17, :], 1.0)
            for k in range(NCHUNK):
                nc.vector.tensor_copy(hf[0:16, OFFS[k]:OFFS[k] + CBS[k]],
                                      S[k][32:48, :])
            PF = ppf.tile([1, BL], F32, tag="PF")
            nc.tensor.matmul(PF[0:1, :], wfc[0:17, 0:1], hf[0:17, :],
                             start=True, stop=True)
            nc.scalar.copy(ys[0:1, :], PF[0:1, :])
            nc.sync.dma_start(y_d.ap(), ys[0:1, :])

    nc.compile()
    return nc


def prep_chunk_inputs(inputs, core, k):
    NPC = np.float32 if _CDT_ENV == "f32" else NPF16
    CB = CBS[k]
    b0 = core * BL + OFFS[k]
    b1 = b0 + CB
    tr = _t_run()
    nblk = (tr + 63) // 64

    x = np.asarray(inputs["x"])[b0:b1]          # [CB, T, IN]
    h0 = np.asarray(inputs["h0"])[:, b0:b1]     # [L, CB, H]
    c0 = np.asarray(inputs["c0"])[:, b0:b1]

    # xt layout: partition = feature, free = t*CB + b (step-major)
    xt = np.zeros((2, nblk * 64 * CB), np.float32)
    xr = x[:, :tr, :].transpose(2, 1, 0)         # [f, t, b]
    xt[:, :tr * CB] = xr.reshape(2, tr * CB)

    s0 = np.zeros((KW, CB), np.float32)
    s0[0:16] = h0[0].T
    s0[16:32] = h0[1].T
    s0[32:48] = h0[2].T
    s0[48] = 1.0
    s0[X0:X0 + 2] = x[:, 0, :].T

    # cell state is stored as c/2 on-device (C-half trick)
    c0p = 0.5 * np.concatenate([c0[0].T, c0[1].T, c0[2].T], axis=0)  # [48, CB]

    return {
        f"xt{k}": xt.astype(NPF16),
        f"s0{k}": s0.astype(NPF16),
        f"c0{k}": np.ascontiguousarray(c0p).astype(NPC),
        f"h1i{k}": np.ascontiguousarray(h0[1].T).astype(NPF16),
        f"c1i{k}": np.ascontiguousarray(0.5 * c0[1].T).astype(NPC),
        f"h2i{k}": np.ascontiguousarray(h0[2].T).astype(NPF16),
        f"c2i{k}": np.ascontiguousarray(0.5 * c0[2].T).astype(NPC),
    }


_NC_CACHE = {}


def kernel(**inputs):
    key = (_t_run(), _CDT_ENV, _FW_POOL, NCHUNK)
    if key not in _NC_CACHE:
        _NC_CACHE[key] = build_bass()
    nc = _NC_CACHE[key]

    b0v = np.asarray(inputs["b_ih0"]) + np.asarray(inputs["b_hh0"])
    b1v = np.asarray(inputs["b_ih1"]) + np.asarray(inputs["b_hh1"])
    b2v = np.asarray(inputs["b_ih2"]) + np.asarray(inputs["b_hh2"])
    WT = build_weight_block(
        np.asarray(inputs["W_ih0"]), np.asarray(inputs["W_hh0"]), b0v,
        np.asarray(inputs["W_ih1"]), np.asarray(inputs["W_hh1"]), b1v,
        np.asarray(inputs["W_ih2"]), np.asarray(inputs["W_hh2"]), b2v,
    ).astype(NPF16)
    wfc = np.zeros((17, 1), np.float32)
    wfc[0:16, 0] = np.asarray(inputs["W_fc"])[0]
    wfc[16, 0] = np.asarray(inputs["b_fc"])[0]
    wfc = wfc.astype(NPF16)

    in_maps = []
    for core in range(NCORES):
        m = {"wt": WT, "wfc": wfc}
        for k in range(NCHUNK):
            m.update(prep_chunk_inputs(inputs, core, k))
        in_maps.append(m)

    trace = os.environ.get("LSTM_TRACE", "0") == "1"
    res = bass_utils.run_bass_kernel_spmd(nc, in_maps, core_ids=list(range(NCORES)),
                                          trace=trace)
    global _LAST_RESULT
    _LAST_RESULT = res
    out = np.concatenate([res.results[c]["y"][0] for c in range(NCORES)])
    return out.reshape(B, 1).astype(np.float32)


_LAST_RESULT = None


if __name__ == "__main__":
    import reference
    inputs = reference.setup_inputs()
    y = kernel(**{k: np.asarray(v) for k, v in inputs.items()})
    print("kernel out", y.shape, y[:4, 0])


# revision 30
# speedup vs baseline: 9.4496x; 1.3751x over previous
"""Trainium2 Bass kernel for a 3-layer LSTM (B=4096, T=1024, IN=2, H=16) + final FC.

Per core (batch-sharded 8 ways, B_local=512), wavefront over layers:
macro-step s computes L0@t=s, L1@t=s-1, L2@t=s-2. The batch is further
split into NCHUNK=3 staggered chunks (172/170/170) so independent
recurrence chains interleave across engines; 3 chunks is the optimum of
the chain-latency (NCHUNK=2) vs ACT-throughput (NCHUNK=4) frontier.

Design notes (vs the 4.29ms DMA-staged baseline; sim ~3.05ms):
  - x lives in DRAM step-major [2, T*CB]; 64-step blocks are DMA-prefetched
    into double-buffered SBUF staging tiles; each step a small Pool-engine
    copy (base 64 -> 64, 32-aligned as the BIR verifier requires of engine
    APs) stages x into the moving window S. No per-step DMA, nothing on the
    recurrence critical path.
  - ONE merged sigmoid per chunk-step over both PSUM banks (FD=2*CB).
  - Cell state is stored as c/2 ("C-half"): c'/2 = HU + FW is a plain
    2x-mode tensor_tensor ADD (scalar_tensor_tensor has no DVE accel mode),
    and tanh(c') = tanh(2 * C_half) uses ACT's free scale=2 input affine.
  - FW and the x-stage copy run on the Pool engine; the DVE keeps only the
    cycle-critical HU -> C' -> h sequence. ACT is the remaining wall
    (~89% busy: sig 468ns + tanh 328ns per 1018ns chunk-slot).

Stationary WT [66, 256] f16: rows 0:48 = h0,h1,h2 recurrent/inter-layer
weights, row 48 = bias, rows 49:64 zero pad, rows 64:66 = x weights
(layer 0). Moving window S [66, CB]: 0:48 h, 48 ones, 64:66 staged x.
Gate cols per 128-bank: layer-l first group (i or 2g) at 16*l, second
(f or o) at 64+16*l; g's weights+bias prescaled x2 so sigmoid(2g) =
(tanh(g)+1)/2 folds tanh into the shared sigmoid table.

Per chunk-step:
  MM_A, MM_B (K=66)                        -> PSUM banks: [i,f] [2g,o]
  SIG  G[128,2,CB] = sigmoid(P)            ACT, one op FD=2*CB
  HU   = (G_2g - 0.5) * G_i                DVE STT  [= i*tanh(g)/2]
  FW   = G_f * C_half                      Pool tensor_mul
  C_half' = HU + FW                        DVE tensor_add (2x mode)
  TC   = tanh(2 * C_half')                 ACT scale=2
  S[0:48] = G_o * TC  (h0,h1,h2 at once)   DVE tensor_mul
"""

import os
import sys

sys.path.insert(0, "/opt/trn_rl_repo")

import numpy as np

import concourse.bacc as bacc
import concourse.mybir as mybir
from concourse.tile import TileContext
from concourse import bass_utils

B, T, IN, H, L = 4096, 1024, 2, 16, 3
NCORES = 8
BL = B // NCORES          # 512
NCHUNK = int(os.environ.get("LSTM_NCHUNK", "3"))
if BL % NCHUNK == 0:
    CBS = [BL // NCHUNK] * NCHUNK
else:
    # uneven chunks (e.g. NCHUNK=3 -> 172,172,168); keep sizes even for
    # DVE 2x mode
    base = (BL // NCHUNK) // 2 * 2
    CBS = [base + 2] * ((BL - base * NCHUNK) // 2)
    CBS += [base] * (NCHUNK - len(CBS))
    assert sum(CBS) == BL and all(c % 2 == 0 for c in CBS), CBS
OFFS = [sum(CBS[:k]) for k in range(NCHUNK)]
F32 = mybir.dt.float32
F16 = mybir.dt.float16
NPF16 = np.float16

# PyTorch gate rows in W_ih*/W_hh*: i, f, g, o
PT_I, PT_F, PT_G, PT_O = slice(0, 16), slice(16, 32), slice(32, 48), slice(48, 64)

_STEPS_ENV = int(os.environ.get("LSTM_STEPS", "0"))
_FW_POOL = os.environ.get("LSTM_FW_POOL", "1") == "1"
_CDT_ENV = os.environ.get("LSTM_CDT", "f16")  # cell-state dtype: f32 | f16


def _t_run():
    return _STEPS_ENV if _STEPS_ENV > 0 else T


KH = 49  # h rows + bias row
X0 = 64  # x rows base: must be 32-aligned for engine (DVE) partition access
KW = 66  # total stationary/moving contraction rows (h + bias + pad + 2 x)


def build_weight_block(W_ih0, W_hh0, b0, W_ih1, W_hh1, b1, W_ih2, W_hh2, b2):
    """WT [51, 256] f32. Cols 0:128 = bank A (i,f), 128:256 = bank B (2g, o).

    K rows: 0:16 h0, 16:32 h1, 32:48 h2, 48 one(bias), 49:51 x.
    Col layout within each bank: layer-l gate block at 16*l : 16*l+16 for
    the first gate group (i or g), 64+16*l : 64+16*l+16 for the second (f or o).
    """
    WT = np.zeros((KW, 256), np.float32)
    layers = [
        # (in_rows, rec_rows, W_ih, W_hh, b)
        (slice(X0, X0 + 2), slice(0, 16), W_ih0, W_hh0, b0),
        (slice(0, 16), slice(16, 32), W_ih1, W_hh1, b1),
        (slice(16, 32), slice(32, 48), W_ih2, W_hh2, b2),
    ]
    for half, (pt0, pt1) in ((0, (PT_I, PT_F)), (128, (PT_G, PT_O))):
        for l, (ir, rr, Wih, Whh, b) in enumerate(layers):
            for pt, cbase, sc in ((pt0, half + 16 * l, 2.0 if half else 1.0),
                                  (pt1, half + 64 + 16 * l, 1.0)):
                WT[ir, cbase:cbase + 16] = sc * Wih[pt].T
                WT[rr, cbase:cbase + 16] = sc * Whh[pt].T
                WT[48, cbase:cbase + 16] = sc * b[pt]
    return WT


def build_bass():
    CDT = F32 if _CDT_ENV == "f32" else F16
    NPC = np.float32 if _CDT_ENV == "f32" else NPF16
    nc = bacc.Bacc("TRN2", target_bir_lowering=False, debug=False,
                   num_devices=NCORES)
    NT = _t_run() + 2
    nblk = (_t_run() + 63) // 64

    wt_d = nc.dram_tensor("wt", [KW, 256], F16, kind="ExternalInput")
    wfc_d = nc.dram_tensor("wfc", [17, 1], F16, kind="ExternalInput")
    xt_d, s0_d, c0_d, h1i_d, c1i_d, h2i_d, c2i_d = [], [], [], [], [], [], []
    for k in range(NCHUNK):
        CB = CBS[k]
        xt_d.append(nc.dram_tensor(f"xt{k}", [2, nblk * 64 * CB], F16,
                                   kind="ExternalInput"))
        s0_d.append(nc.dram_tensor(f"s0{k}", [KW, CB], F16,
                                   kind="ExternalInput"))
        c0_d.append(nc.dram_tensor(f"c0{k}", [48, CB],
                                   mybir.dt.from_np(np.dtype(NPC)),
                                   kind="ExternalInput"))
        h1i_d.append(nc.dram_tensor(f"h1i{k}", [16, CB], F16,
                                    kind="ExternalInput"))
        c1i_d.append(nc.dram_tensor(f"c1i{k}", [16, CB],
                                    mybir.dt.from_np(np.dtype(NPC)),
                                    kind="ExternalInput"))
        h2i_d.append(nc.dram_tensor(f"h2i{k}", [16, CB], F16,
                                    kind="ExternalInput"))
        c2i_d.append(nc.dram_tensor(f"c2i{k}", [16, CB],
                                    mybir.dt.from_np(np.dtype(NPC)),
                                    kind="ExternalInput"))
    y_d = nc.dram_tensor("y", [1, BL], F32, kind="ExternalOutput")

    SIG = mybir.ActivationFunctionType.Sigmoid
    TANH = mybir.ActivationFunctionType.Tanh
    ADD = mybir.AluOpType.add
    MULT = mybir.AluOpType.mult

    with TileContext(nc) as tc:
        wt = nc.alloc_sbuf_tensor("wt_sb", [KW, 256], F16)
        wfc = nc.alloc_sbuf_tensor("wfc_sb", [17, 1], F16)
        hf = nc.alloc_sbuf_tensor("hf_sb", [17, BL], F16)
        ys = nc.alloc_sbuf_tensor("ys_sb", [1, BL], F32)
        # x staging: two 64-step blocks per chunk, rows 49:51 so the
        # per-step DVE copy into S[49:51] is a zero-shift partition access
        xtb, S, C = [], [], []
        for k in range(NCHUNK):
            CB = CBS[k]
            xtb.append([nc.alloc_sbuf_tensor(f"xtb{k}_{j}", [KW, 64 * CB], F16)
                        for j in range(2)])
            S.append(nc.alloc_sbuf_tensor(f"S_sb{k}", [KW, CB], F16))
            C.append(nc.alloc_sbuf_tensor(f"C_sb{k}", [112, CB], CDT))

        def xblk(k, blk):
            """Prefetch 64-step x block `blk` into its staging buffer."""
            CB = CBS[k]
            n0 = blk * 64 * CB
            nc.sync.dma_start(xtb[k][blk % 2][X0:KW, :],
                              xt_d[k].ap()[0:2, n0:n0 + 64 * CB])

        nc.sync.dma_start(wt[:, :], wt_d.ap())
        nc.sync.dma_start(wfc[:, :], wfc_d.ap())
        for k in range(NCHUNK):
            nc.sync.dma_start(S[k][:, :], s0_d[k].ap())
            nc.sync.dma_start(C[k][64:112, :], c0_d[k].ap())
            xblk(k, 0)
            if nblk > 1:
                xblk(k, 1)

        psum_bufs = 1 if sum(2 * cb * 4 * 2 for cb in CBS) > 16384 else 2
        fw_eng = nc.gpsimd if _FW_POOL else nc.vector
        with tc.tile_pool(name="ps", bufs=psum_bufs, space="PSUM") as pps, \
             tc.tile_pool(name="sb", bufs=4) as psb:
            # Emission order = scheduler priority (tie-break among ready
            # ops). Interleave the two chunks at op granularity so chunk b's
            # front fills chunk a's C'->tanh latency gap.
            live = [None] * NCHUNK  # per-chunk (G, HU, FW)

            # Optional static phase-pinning: LSTM_LAM pins a steady-state
            # period (ns); each op class gets a per-macro lower-bound
            # timestamp so the greedy scheduler follows a fixed software
            # pipeline instead of making myopic choices. 0 = off.
            LAM = float(os.environ.get("LSTM_LAM", "0"))
            # op-class offsets within a chunk's cycle (ns from MM issue)
            PH_MM, PH_SIG, PH_MID, PH_TANH, PH_H = 0.0, 293.0, 880.0, 1540.0, 1990.0

            from contextlib import nullcontext

            def pin(w, off):
                if LAM <= 0 or w is None:
                    return nullcontext()
                return tc.tile_wait_until((w + off) * 1e-6)

            def front(k, m, w=None):
                CB = CBS[k]
                P = pps.tile([128, 2, CB], F32, tag=f"P{k}")
                G = psb.tile([128, 2, CB], F16, tag=f"G{k}")
                HU = psb.tile([48, CB], F16, tag=f"HU{k}")
                FW = psb.tile([48, CB], CDT, tag=f"FW{k}")
                with pin(w, PH_MM):
                    # K=66 contraction: h rows, bias row, zero pad, x rows.
                    nc.tensor.matmul(P[0:128, 0, 0:CB], wt[0:KW, 0:128],
                                     S[k][0:KW, :], start=True, stop=True)
                    nc.tensor.matmul(P[0:128, 1, 0:CB], wt[0:KW, 128:256],
                                     S[k][0:KW, :], start=True, stop=True)
                with pin(w, PH_SIG):
                    # One sigmoid across both banks: i,f (bank0) + 2g,o (b1)
                    nc.scalar.activation(G[0:128, 0:2, 0:CB],
                                         P[0:128, 0:2, 0:CB], SIG)
                live[k] = (G, HU, FW)

            _MID_HU_FIRST = os.environ.get("LSTM_HU_FIRST", "1") == "1"

            def mid(k, w=None):
                CB = CBS[k]
                G, HU, FW = live[k]
                with pin(w, PH_MID):
                    # HU first: FW then fills the scheduler's 117ns
                    # same-engine readiness gap before C' instead of an
                    # unrelated op wedging in and delaying tanh.
                    if _MID_HU_FIRST:
                        nc.vector.scalar_tensor_tensor(
                            HU[0:48, :], G[0:48, 1, 0:CB], -0.5,
                            G[0:48, 0, 0:CB], ADD, MULT)
                        fw_eng.tensor_mul(FW[0:48, :], G[64:112, 0, 0:CB],
                                          C[k][64:112, :])
                    else:
                        fw_eng.tensor_mul(FW[0:48, :], G[64:112, 0, 0:CB],
                                          C[k][64:112, :])
                        nc.vector.scalar_tensor_tensor(
                            HU[0:48, :], G[0:48, 1, 0:CB], -0.5,
                            G[0:48, 0, 0:CB], ADD, MULT)

            def cprime(k, w=None):
                G, HU, FW = live[k]
                TC = psb.tile([112, CBS[k]], F16, tag=f"TC{k}")
                with pin(w, PH_MID):
                    # c_half' = hu + fw   (plain TT add -> 2x mode)
                    nc.vector.tensor_add(C[k][64:112, :], HU[0:48, :],
                                         FW[0:48, :])
                with pin(w, PH_TANH):
                    # tc = tanh(2 * c_half') = tanh(c')
                    nc.scalar.activation(TC[64:112, :], C[k][64:112, :], TANH,
                                         scale=2.0)
                live[k] = (G, TC)

            def hout(k, s, w=None):
                CB = CBS[k]
                G, TC = live[k]
                with pin(w, PH_H):
                    # h0,h1,h2 = o * tc in one op
                    nc.vector.tensor_mul(S[k][0:48, :], G[64:112, 1, 0:CB],
                                         TC[64:112, :])
                    # delayed init: overwrite wavefront-startup pollution
                    if s == 0:
                        nc.sync.dma_start(S[k][16:32, :], h1i_d[k].ap())
                        nc.sync.dma_start(C[k][80:96, :], c1i_d[k].ap())
                    elif s == 1:
                        nc.sync.dma_start(S[k][32:48, :], h2i_d[k].ap())
                        nc.sync.dma_start(C[k][96:112, :], c2i_d[k].ap())

            xc_eng = nc.gpsimd if os.environ.get("LSTM_XC_POOL", "1") == "1" \
                else nc.vector

            def xstage(k, s):
                # stage next x: copy from the current 64-step staging block
                # (zero partition shift, base 64 -> legal on Pool too).
                # Pool keeps it off the DVE queue entirely; emitted at the
                # END of the macro's stream so its priority ranks below
                # every cycle-critical op. Correctness is unaffected: Tile
                # orders it after this step's MM reads of S (WAR) and
                # before the next step's (RAW).
                CB = CBS[k]
                if s + 1 < _t_run():
                    nb_, nu = divmod(s + 1, 64)
                    xc_eng.tensor_copy(
                        S[k][X0:KW, :],
                        xtb[k][nb_ % 2][X0:KW, nu * CB:(nu + 1) * CB])
                    # one step into block nb_, its predecessor buffer is
                    # free: prefetch block nb_+1 into it
                    if nu == 1 and nb_ + 1 < nblk:
                        xblk(k, nb_ + 1)

            def wbase(m, k):
                if LAM <= 0:
                    return None
                return m * LAM + k * LAM / NCHUNK

            for m in range(NT):
                if NCHUNK == 2:
                    front(0, m, wbase(m, 0))
                    front(1, m, wbase(m, 1))
                    mid(0, wbase(m, 0))
                    cprime(0, wbase(m, 0))
                    mid(1, wbase(m, 1))
                    hout(0, m, wbase(m, 0))
                    cprime(1, wbase(m, 1))
                    hout(1, m, wbase(m, 1))
                else:
                    for k in range(NCHUNK):
                        front(k, m, wbase(m, k))
                    mid(0, wbase(m, 0))
                    cprime(0, wbase(m, 0))
                    for k in range(1, NCHUNK):
                        mid(k, wbase(m, k))
                        hout(k - 1, m, wbase(m, k - 1))
                        cprime(k, wbase(m, k))
                    hout(NCHUNK - 1, m, wbase(m, NCHUNK - 1))
                for k in range(NCHUNK):
                    xstage(k, m)

        # final fc: y = h2 @ W_fc.T + b_fc
        with tc.tile_pool(name="pf", bufs=1, space="PSUM") as ppf:
            nc.vector.memset(hf[0:17, :], 1.0)
            for k in range(NCHUNK):
                nc.vector.tensor_copy(hf[0:16, OFFS[k]:OFFS[k] + CBS[k]],
                                      S[k][32:48, :])
            PF = ppf.tile([1, BL], F32, tag="PF")
            nc.tensor.matmul(PF[0:1, :], wfc[0:17, 0:1], hf[0:17, :],
                             start=True, stop=True)
            nc.scalar.copy(ys[0:1, :], PF[0:1, :])
            nc.sync.dma_start(y_d.ap(), ys[0:1, :])

    nc.compile()
    return nc


def prep_chunk_inputs(inputs, core, k):
    NPC = np.float32 if _CDT_ENV == "f32" else NPF16
    CB = CBS[k]
    b0 = core * BL + OFFS[k]
    b1 = b0 + CB
    tr = _t_run()
    nblk = (tr + 63) // 64

    x = np.asarray(inputs["x"])[b0:b1]          # [CB, T, IN]
    h0 = np.asarray(inputs["h0"])[:, b0:b1]     # [L, CB, H]
    c0 = np.asarray(inputs["c0"])[:, b0:b1]

    # xt layout: partition = feature, free = t*CB + b (step-major)
    xt = np.zeros((2, nblk * 64 * CB), np.float32)
    xr = x[:, :tr, :].transpose(2, 1, 0)         # [f, t, b]
    xt[:, :tr * CB] = xr.reshape(2, tr * CB)

    s0 = np.zeros((KW, CB), np.float32)
    s0[0:16] = h0[0].T
    s0[16:32] = h0[1].T
    s0[32:48] = h0[2].T
    s0[48] = 1.0
    s0[X0:X0 + 2] = x[:, 0, :].T

    # cell state is stored as c/2 on-device (C-half trick)
    c0p = 0.5 * np.concatenate([c0[0].T, c0[1].T, c0[2].T], axis=0)  # [48, CB]

    return {
        f"xt{k}": xt.astype(NPF16),
        f"s0{k}": s0.astype(NPF16),
        f"c0{k}": np.ascontiguousarray(c0p).astype(NPC),
        f"h1i{k}": np.ascontiguousarray(h0[1].T).astype(NPF16),
        f"c1i{k}": np.ascontiguousarray(0.5 * c0[1].T).astype(NPC),
        f"h2i{k}": np.ascontiguousarray(h0[2].T).astype(NPF16),
        f"c2i{k}": np.ascontiguousarray(0.5 * c0[2].T).astype(NPC),
    }


_NC_CACHE = {}


def kernel(**inputs):
    key = (_t_run(), _CDT_ENV, _FW_POOL, NCHUNK)
    if key not in _NC_CACHE:
        _NC_CACHE[key] = build_bass()
    nc = _NC_CACHE[key]

    b0v = np.asarray(inputs["b_ih0"]) + np.asarray(inputs["b_hh0"])
    b1v = np.asarray(inputs["b_ih1"]) + np.asarray(inputs["b_hh1"])
    b2v = np.asarray(inputs["b_ih2"]) + np.asarray(inputs["b_hh2"])
    WT = build_weight_block(
        np.asarray(inputs["W_ih0"]), np.asarray(inputs["W_hh0"]), b0v,
        np.asarray(inputs["W_ih1"]), np.asarray(inputs["W_hh1"]), b1v,
        np.asarray(inputs["W_ih2"]), np.asarray(inputs["W_hh2"]), b2v,
    ).astype(NPF16)
    wfc = np.zeros((17, 1), np.float32)
    wfc[0:16, 0] = np.asarray(inputs["W_fc"])[0]
    wfc[16, 0] = np.asarray(inputs["b_fc"])[0]
    wfc = wfc.astype(NPF16)

    in_maps = []
    for core in range(NCORES):
        m = {"wt": WT, "wfc": wfc}
        for k in range(NCHUNK):
            m.update(prep_chunk_inputs(inputs, core, k))
        in_maps.append(m)

    trace = os.environ.get("LSTM_TRACE", "0") == "1"
    res = bass_utils.run_bass_kernel_spmd(nc, in_maps, core_ids=list(range(NCORES)),
                                          trace=trace)
    global _LAST_RESULT
    _LAST_RESULT = res
    out = np.concatenate([res.results[c]["y"][0] for c in range(NCORES)])
    return out.reshape(B, 1).astype(np.float32)


_LAST_RESULT = None


if __name__ == "__main__":
    import reference
    inputs = reference.setup_inputs()
    y = kernel(**{k: np.asarray(v) for k, v in inputs.items()})
    print("kernel out", y.shape, y[:4, 0])
